# revision 11
# baseline (speedup 1.0000x reference)
"""AdaptiveKernelFC Trainium2 kernel (8-core data parallel), v2.

Math: the reference builds per-sample filters w[n,p,c,kh,kw] =
x[n,c,kh,kw]*Wk[p] + bk[p] and convolves x[n] with them (7x7 kernel ==
feature map size, pad 3).  The conv factors exactly:

    y[n,p,i,j] = Wk[p]*S1[n,i,j] + bk[p]*S2[n,i,j] + b_adap[p]

with S1 the 2D autocorrelation of x[n] (summed over channels) and S2
the 7x7 box-filter correlation of the channel sums.  Both come from one
Gram-style matmul per sample: stationary [x | ones] interleaved in
groups of 14 (7 x-columns of spatial row a, then 7 ones columns), so
PSUM partition p = 14a+7w+b holds, over free columns q:
    w=0:  G[(a,b), q]   (spatial Gram row)
    w=1:  xs[q]         (channel sums, replicated)

The diagonal band T[p, n, t] = row p's value at column (7a+b-24)+t is
gathered via a DRAM staging round trip (SBUF DMAs cannot mix partition
and element steps; DRAM is flat): one dump at row stride 256, then 7
comb gathers (one per b) with legal 3-dim APs.  Pad/guard regions of
the staging buffer are zeroed up front so out-of-band reads are finite;
a single mask-multiply (f32 mask, bf16 output) zeroes the wrapped
positions and casts for the TensorEngine in one op.

The final stage contracts T directly against Q (98, 256) where
Q[p, :] = Wk if w=0 else bk -- built by two replication DMAs -- so the
selector matmul, bias adds and separate R staging all disappear:
    y[ck*128+p, (n,s)] = (Q[:, ck]^T T)[p, (n,s)] + b_adap  (2 matmuls,
    bias added by the PSUM->SBUF tensor_scalar_add).

Sharding: pure data parallel, batch N=32 split 4 samples/core across 8
cores; params replicated; outputs concatenated.
"""

import os
import numpy as np

import concourse.bass as bass
import concourse.bacc as bacc
import concourse.mybir as mybir
import concourse.tile as tile
from concourse.ap import AP
from concourse.bass_utils import run_bass_kernel_spmd

N, C, H, W = 32, 256, 7, 7
P = 256
NCORES = 8
B = N // NCORES          # samples per core
HW = H * W               # 49
ROWS = 2 * HW            # 98 PSUM rows (interleaved G / xs)
FREE = B * HW            # 196 free columns (n, t)
SROW = 256               # staging row stride (f32 elems)
F32 = mybir.dt.float32
BF16 = mybir.dt.bfloat16

_cached = {}
last_exec_time_ns = None


def _mask_np():
    # mask[p, t]: p = 14a+7w+b, t = 7si+sj; valid iff the shifted window
    # position stays on the 7x7 grid in both axes.
    m = np.zeros((ROWS, HW), dtype=np.float32)
    for p in range(ROWS):
        a, v = divmod(p, 14)
        b = v % 7
        for t in range(HW):
            si, sj = divmod(t, 7)
            if 0 <= a + si - 3 < 7 and 0 <= b + sj - 3 < 7:
                m[p, t] = 1.0
    return m


def build():
    nc = bacc.Bacc(
        "TRN2", target_bir_lowering=False, debug=False, num_devices=NCORES
    )
    x_d = nc.dram_tensor("x", (B, C, H, W), F32, kind="ExternalInput")
    wk_d = nc.dram_tensor("Wk", (P,), F32, kind="ExternalInput")
    bk_d = nc.dram_tensor("bk", (P,), F32, kind="ExternalInput")
    ba_d = nc.dram_tensor("b_adap", (P,), F32, kind="ExternalInput")
    out_d = nc.dram_tensor("out", (B, P, H, W), F32, kind="ExternalOutput")
    stag_d = nc.dram_tensor("stag", (SROW * (ROWS + 2),), F32, kind="Internal")

    with tile.TileContext(nc) as tc:
        with (
            tc.tile_pool(name="sb", bufs=1) as sb,
            tc.tile_pool(name="ps", bufs=1, space="PSUM") as ps,
        ):
            xsb = sb.tile([128, 2, B, HW], F32)       # x, channels on partitions
            xbf = sb.tile([128, 2, B, ROWS], BF16)    # [x | ones] interleaved by 14
            gsb = sb.tile([ROWS, FREE], F32)          # PSUM rows staged for dump
            T = sb.tile([ROWS, FREE], F32)            # gathered bands
            Tbf = sb.tile([ROWS, FREE], BF16)         # masked + cast
            mk = sb.tile([ROWS, HW], F32)             # band validity mask
            prm = sb.tile([2, P], F32)                # Wk; bk
            prmbf = sb.tile([2, P], BF16)
            selT = sb.tile([2, ROWS], BF16)           # row-kind selector
            Qbf = sb.tile([ROWS, P], BF16)
            badap = sb.tile([128, 2], F32)            # b_adap, chunked
            zsb = sb.tile([ROWS, 64], F32)            # zero source for guards
            ysb = sb.tile([128, 2, FREE], F32)

            GX_ps = ps.tile([ROWS, FREE], F32)
            Q_ps = ps.tile([ROWS, P], F32)
            Y_ps = [ps.tile([128, FREE], F32, name=f"y{k}") for k in range(2)]

            import ml_dtypes

            sel_np = np.zeros((2, ROWS), dtype=ml_dtypes.bfloat16)
            for p in range(ROWS):
                sel_np[(p % 14) // 7, p] = 1.0
            sel_d = nc.inline_tensor(sel_np, name="sel_const")
            mask_d = nc.inline_tensor(_mask_np(), name="mask_const")
            nc.gpsimd.dma_start(mk[:], mask_d[:])

            # ones everywhere; x casts below overwrite the x column groups
            nc.vector.memset(xbf[:], 1.0)
            nc.vector.memset(zsb[:], 0.0)

            # staging guards: zero the in-row pads and the 2 tail rows so
            # out-of-band gather reads are finite (masked later)
            nc.gpsimd.dma_start(
                AP(stag_d, 0, [[SROW, ROWS], [1, 24]]), zsb[:, 0:24]
            )
            nc.gpsimd.dma_start(
                AP(stag_d, 24 + FREE, [[SROW, ROWS], [1, 36]]), zsb[:, 24:60]
            )
            nc.gpsimd.dma_start(
                AP(stag_d, SROW * ROWS, [[64, 8], [1, 64]]), zsb[0:8, 0:64]
            )

            # x -> SBUF with channels on partitions; 4 loads on 4 queues
            xr = x_d.ap().rearrange("n (k c) h w -> k c n (h w)", k=2)
            nc.sync.dma_start(xsb[:, 0, 0:2], xr[0, :, 0:2])
            nc.scalar.dma_start(xsb[:, 1, 0:2], xr[1, :, 0:2])
            nc.sync.dma_start(xsb[:, 0, 2:4], xr[0, :, 2:4])
            nc.scalar.dma_start(xsb[:, 1, 2:4], xr[1, :, 2:4])

            # f32 -> bf16 into the interleaved x column groups (a, 0:7)
            for ck in range(2):
                for ns in (slice(0, 2), slice(2, 4)):
                    nc.vector.tensor_copy(
                        xbf[:, ck, ns].rearrange("p n (g j) -> p n g j", j=14)[
                            :, :, :, 0:7
                        ],
                        xsb[:, ck, ns].rearrange("p n (a b) -> p n a b", b=7),
                    )

            # Q: Wk on w=0 rows, bk on w=1 rows via tiny selector matmul
            nc.gpsimd.dma_start(prm[0:1, :], wk_d.ap().unsqueeze(0))
            nc.gpsimd.dma_start(prm[1:2, :], bk_d.ap().unsqueeze(0))
            nc.vector.tensor_copy(prmbf[:], prm[:])
            nc.gpsimd.dma_start(selT[:], sel_d[:])
            nc.tensor.matmul(Q_ps[:], selT[:], prmbf[:], start=True, stop=True)
            nc.vector.tensor_copy(Qbf[:], Q_ps[:])
            nc.gpsimd.dma_start(badap[:], AP(ba_d, 0, [[1, 128], [128, 2]]))

            # Gram + channel sums: contract channels in 2 chunks per sample
            for n in range(B):
                xn = [
                    xbf[:, ck, n].rearrange("p (g j) -> p g j", j=14)[:, :, 0:7]
                    for ck in range(2)
                ]
                for ck in range(2):
                    nc.tensor.matmul(
                        GX_ps[:, n * HW : (n + 1) * HW],
                        xbf[:, ck, n],
                        xn[ck],
                        start=(ck == 0),
                        stop=(ck == 1),
                    )

            # PSUM -> SBUF, then dump all 98 rows at stride 256 (col 24)
            nc.vector.tensor_copy(gsb[:], GX_ps[:])
            nc.sync.dma_start(AP(stag_d, 24, [[SROW, ROWS], [1, FREE]]), gsb[:])

            # 14 comb gathers: T[p=14a+7w+b, (n,t)] = stag[256p + 24 + (7a+b-24) + n*49 + t]
            #                                       = stag[(14*256+7)a + (7*256)w + 257b + (n,t)]
            # (2-dim APs only: 3-dim SBUF dst with two partition-crossing
            # dims lowers incorrectly -- see probe2)
            for b in range(7):
                for w in range(2):
                    src = AP(
                        stag_d,
                        257 * b + 7 * SROW * w,
                        [[14 * SROW + 7, 7], [1, FREE]],
                    )
                    dst = AP(
                        T.tensor,
                        FREE * b + 7 * FREE * w,
                        [[14 * FREE, 7], [1, FREE]],
                    )
                    (nc.scalar if (2 * b + w) % 2 else nc.sync).dma_start(dst, src)

            # mask (broadcast over n) * T -> bf16 in one DVE op
            mkb = AP(mk.tensor, 0, [[HW, ROWS], [0, B], [1, HW]])
            tv = T[:].rearrange("p (n t) -> p n t", n=B)
            nc.vector.tensor_tensor(
                Tbf[:].rearrange("p (n t) -> p n t", n=B),
                tv,
                mkb,
                op=mybir.AluOpType.mult,
            )

            # final matmuls + bias via tensor_scalar_add during PSUM->SBUF
            outr = out_d.ap().rearrange("n (k p) h w -> k p n (h w)", k=2)
            for pk in range(2):
                nc.tensor.matmul(
                    Y_ps[pk][:],
                    Qbf[:, pk * 128 : (pk + 1) * 128],
                    Tbf[:],
                    start=True,
                    stop=True,
                )
                nc.vector.tensor_scalar_add(
                    ysb[:, pk], Y_ps[pk][:], badap[:, pk : pk + 1]
                )
            nc.sync.dma_start(outr[0][:, 0:2], ysb[:, 0, 0 : 2 * HW].rearrange("p (n s) -> p n s", n=2))
            nc.scalar.dma_start(outr[0][:, 2:4], ysb[:, 0, 2 * HW :].rearrange("p (n s) -> p n s", n=2))
            nc.sync.dma_start(outr[1][:, 0:2], ysb[:, 1, 0 : 2 * HW].rearrange("p (n s) -> p n s", n=2))
            nc.scalar.dma_start(outr[1][:, 2:4], ysb[:, 1, 2 * HW :].rearrange("p (n s) -> p n s", n=2))

    nc.compile()
    return nc


def kernel(x, Wk, bk, b_adap):
    global last_exec_time_ns
    if "nc" not in _cached:
        _cached["nc"] = build()
    nc = _cached["nc"]

    x = np.ascontiguousarray(x, dtype=np.float32)
    Wk = np.ascontiguousarray(Wk, dtype=np.float32)
    bk = np.ascontiguousarray(bk, dtype=np.float32)
    b_adap = np.ascontiguousarray(b_adap, dtype=np.float32)

    in_maps = [
        {"x": x[i * B : (i + 1) * B], "Wk": Wk, "bk": bk, "b_adap": b_adap}
        for i in range(NCORES)
    ]
    res = run_bass_kernel_spmd(
        nc,
        in_maps,
        core_ids=list(range(NCORES)),
        trace=bool(os.environ.get("KERNEL_TRACE")),
    )
    last_exec_time_ns = res.exec_time_ns
    out = np.concatenate(
        [res.results[i]["out"].reshape(B, P, H, W) for i in range(NCORES)], axis=0
    )
    return out


# revision 12
# speedup vs baseline: 1.2361x; 1.2361x over previous
"""AdaptiveKernelFC Trainium2 kernel (8-core data parallel), v3.

Math: the reference builds per-sample filters w[n,p,c,kh,kw] =
x[n,c,kh,kw]*Wk[p] + bk[p] and convolves x[n] with them (7x7 kernel ==
feature map size, pad 3).  The conv factors exactly:

    y[n,p,i,j] = Wk[p]*S1[n,i,j] + bk[p]*S2[n,i,j] + b_adap[p]

with S1 the 2D autocorrelation of x[n] (summed over channels) and S2
the 7x7 box-filter correlation of the channel sums.  Both come from one
fused matmul pair per sample with stationary [x | ones]: PSUM rows 0:49
hold the spatial Gram matrix G[r, q], rows 49:98 hold the channel sums
xs[q] replicated.

The diagonal band T[p, (n, t)] = row p's value at column (r-24)+t
(r = p mod 49) is produced by a DRAM staging round trip (SBUF DMAs
cannot mix partition and element steps; DRAM is flat): dump rows at
stride 192 starting at column 24, then ONE gather per (region, half)
with row stride 193 -- the +1 slope realizes the per-row shift.  The
staging tensors are inline zero constants, so the out-of-band positions
the gather sweeps through are always finite zeros/neighbor data; a
single mask-multiply (f32 mask broadcast over samples, bf16 output)
zeroes the wrapped positions and casts for the TensorEngine in one op.

The final stage contracts T directly against Q (98, 256) where
Q[p, :] = Wk if p < 49 else bk (built once off the critical path by a
tiny selector matmul), so the selector reduction, bias staging and R
assembly of earlier versions all disappear:

    y[ck*128+p, (n,s)] = (Q[:, ck]^T Tbf)[p, (n,s)] + b_adap
    (b_adap added by the PSUM->SBUF tensor_scalar_add move).

Everything is split into two sample-halves so the second half's Gram
matmuls and round trip overlap the first half's output pipeline.

Sharding: pure data parallel, batch N=32 split 4 samples/core across 8
cores; params replicated; outputs concatenated.
"""

import os
import numpy as np

import concourse.bass as bass
import concourse.bacc as bacc
import concourse.mybir as mybir
import concourse.tile as tile
from concourse.ap import AP
from concourse.bass_utils import run_bass_kernel_spmd

N, C, H, W = 32, 256, 7, 7
P = 256
NCORES = 8
B = N // NCORES          # samples per core
HW = H * W               # 49
ROWS = 2 * HW            # 98 PSUM rows (G region then xs region)
FREE = B * HW            # 196 free columns (n, t)
HF = 2 * HW              # 98 free columns per half
SROW = 192               # staging row stride (f32 elems); band reads stay in-row
F32 = mybir.dt.float32
BF16 = mybir.dt.bfloat16

_cached = {}
last_exec_time_ns = None


def _mask_np():
    # mask[p, t]: r = p mod 49 = 7a+b, t = 7si+sj; valid iff the shifted
    # window position stays on the 7x7 grid in both axes.
    m = np.zeros((ROWS, HW), dtype=np.float32)
    for p in range(ROWS):
        a, b = divmod(p % HW, 7)
        for t in range(HW):
            si, sj = divmod(t, 7)
            if 0 <= a + si - 3 < 7 and 0 <= b + sj - 3 < 7:
                m[p, t] = 1.0
    return m


def build():
    import ml_dtypes

    nc = bacc.Bacc(
        "TRN2", target_bir_lowering=False, debug=False, num_devices=NCORES
    )
    x_d = nc.dram_tensor("x", (B, C, H, W), F32, kind="ExternalInput")
    wk_d = nc.dram_tensor("Wk", (P,), F32, kind="ExternalInput")
    bk_d = nc.dram_tensor("bk", (P,), F32, kind="ExternalInput")
    ba_d = nc.dram_tensor("b_adap", (P,), F32, kind="ExternalInput")
    out_d = nc.dram_tensor("out", (B, P, H, W), F32, kind="ExternalOutput")
    # staging: inline zero constants -> pads are zero at model load and the
    # per-run dump only ever writes the data region (cols 24:122)
    stag = [
        nc.inline_tensor(
            np.zeros(SROW * ROWS, dtype=np.float32), name=f"stag{nh}"
        )
        for nh in range(2)
    ]

    with tile.TileContext(nc) as tc:
        with (
            tc.tile_pool(name="sb", bufs=1) as sb,
            tc.tile_pool(name="ps", bufs=1, space="PSUM") as ps,
        ):
            xsb = sb.tile([128, 2, B, HW], F32)       # x, channels on partitions
            xbf = sb.tile([128, 2, B, ROWS], BF16)    # [x | ones] per (ck, n)
            gsb = sb.tile([ROWS, 2, HF], F32)         # PSUM rows staged per half
            T = sb.tile([ROWS, 2, HF], F32)           # gathered bands per half
            Tbf = sb.tile([ROWS, 2, HF], BF16)        # masked + cast
            mk = sb.tile([ROWS, HW], F32)             # band validity mask
            prm = sb.tile([2, P], F32)                # Wk; bk
            prmbf = sb.tile([2, P], BF16)
            selT = sb.tile([2, ROWS], BF16)           # row-region selector
            Qbf = sb.tile([ROWS, P], BF16)
            badap = sb.tile([128, 2], F32)            # b_adap, chunked
            ysb = sb.tile([128, 2, 2, HF], F32)       # (p, ck, nh, (n2, t))

            GX_ps = ps.tile([ROWS, FREE], F32)
            Q_ps = ps.tile([ROWS, P], F32)
            Y_ps = [ps.tile([128, FREE], F32, name=f"y{k}") for k in range(2)]

            sel_np = np.zeros((2, ROWS), dtype=ml_dtypes.bfloat16)
            sel_np[0, 0:HW] = 1.0
            sel_np[1, HW:ROWS] = 1.0
            sel_d = nc.inline_tensor(sel_np, name="sel_const")
            mask_d = nc.inline_tensor(_mask_np(), name="mask_const")

            # prologue constants / params on the software DGE (off critical path)
            nc.gpsimd.dma_start(mk[:], mask_d[:])
            nc.gpsimd.dma_start(selT[:], sel_d[:])
            nc.gpsimd.dma_start(prm[0:1, :], wk_d.ap().unsqueeze(0))
            nc.gpsimd.dma_start(prm[1:2, :], bk_d.ap().unsqueeze(0))
            nc.gpsimd.dma_start(badap[:], AP(ba_d, 0, [[1, 128], [128, 2]]))

            # ones region of the stationary; x casts fill cols 0:49
            nc.vector.memset(xbf[:, :, :, HW:ROWS], 1.0)

            # x -> SBUF, channels on partitions; first half first on both queues
            xr = x_d.ap().rearrange("n (k c) h w -> k c n (h w)", k=2)
            nc.sync.dma_start(xsb[:, 0, 0:2], xr[0, :, 0:2])
            nc.scalar.dma_start(xsb[:, 1, 0:2], xr[1, :, 0:2])
            nc.sync.dma_start(xsb[:, 0, 2:4], xr[0, :, 2:4])
            nc.scalar.dma_start(xsb[:, 1, 2:4], xr[1, :, 2:4])

            # Q = selT^T @ [Wk; bk]: rows 0:49 Wk, rows 49:98 bk
            nc.vector.tensor_copy(prmbf[:], prm[:])
            nc.tensor.matmul(Q_ps[:], selT[:], prmbf[:], start=True, stop=True)
            nc.vector.tensor_copy(Qbf[:], Q_ps[:])

            for ck in range(2):
                for nh in range(2):
                    ns = slice(2 * nh, 2 * nh + 2)
                    nc.vector.tensor_copy(xbf[:, ck, ns, 0:HW], xsb[:, ck, ns])

            outr = out_d.ap().rearrange("n (k p) h w -> k p n (h w)", k=2)
            mkb = AP(mk.tensor, 0, [[HW, ROWS], [0, 2], [1, HW]])
            for nh in range(2):
                ns = slice(2 * nh, 2 * nh + 2)
                for n in range(2 * nh, 2 * nh + 2):
                    for ck in range(2):
                        nc.tensor.matmul(
                            GX_ps[:, n * HW : (n + 1) * HW],
                            xbf[:, ck, n],
                            xbf[:, ck, n, 0:HW],
                            start=(ck == 0),
                            stop=(ck == 1),
                        )
                nc.vector.tensor_copy(
                    gsb[:, nh], GX_ps[:, 2 * nh * HW : (2 * nh + 2) * HW]
                )
                (nc.sync if nh == 0 else nc.scalar).dma_start(
                    AP(stag[nh], 24, [[SROW, ROWS], [1, HF]]), gsb[:, nh]
                )
                # one gather per region: T[reg*49+r, nh, i] = stag[nh][193*(reg*49+r) - reg*49*... ]
                # flat: stag[192*p + r + i] with p = reg*49 + r
                for reg in range(2):
                    src = AP(stag[nh], SROW * HW * reg, [[SROW + 1, HW], [1, HF]])
                    dst = AP(
                        T.tensor,
                        (HW * reg) * (2 * HF) + nh * HF,
                        [[2 * HF, HW], [1, HF]],
                    )
                    (nc.sync if reg == 0 else nc.scalar).dma_start(dst, src)
                nc.vector.tensor_tensor(
                    Tbf[:, nh].rearrange("p (n t) -> p n t", n=2),
                    T[:, nh].rearrange("p (n t) -> p n t", n=2),
                    mkb,
                    op=mybir.AluOpType.mult,
                )
                for pk in range(2):
                    nc.tensor.matmul(
                        Y_ps[pk][:, nh * HF : (nh + 1) * HF],
                        Qbf[:, pk * 128 : (pk + 1) * 128],
                        Tbf[:, nh],
                        start=True,
                        stop=True,
                    )
                    nc.vector.tensor_scalar_add(
                        ysb[:, pk, nh],
                        Y_ps[pk][:, nh * HF : (nh + 1) * HF],
                        badap[:, pk : pk + 1],
                    )
                    (nc.sync if pk == 0 else nc.scalar).dma_start(
                        outr[pk][:, ns],
                        ysb[:, pk, nh].rearrange("p (n t) -> p n t", n=2),
                    )

    nc.compile()
    return nc


def kernel(x, Wk, bk, b_adap):
    global last_exec_time_ns
    if "nc" not in _cached:
        _cached["nc"] = build()
    nc = _cached["nc"]

    x = np.ascontiguousarray(x, dtype=np.float32)
    Wk = np.ascontiguousarray(Wk, dtype=np.float32)
    bk = np.ascontiguousarray(bk, dtype=np.float32)
    b_adap = np.ascontiguousarray(b_adap, dtype=np.float32)

    in_maps = [
        {"x": x[i * B : (i + 1) * B], "Wk": Wk, "bk": bk, "b_adap": b_adap}
        for i in range(NCORES)
    ]
    res = run_bass_kernel_spmd(
        nc,
        in_maps,
        core_ids=list(range(NCORES)),
        trace=bool(os.environ.get("KERNEL_TRACE")),
    )
    last_exec_time_ns = res.exec_time_ns
    out = np.concatenate(
        [res.results[i]["out"].reshape(B, P, H, W) for i in range(NCORES)], axis=0
    )
    return out
